# revision 1
# baseline (speedup 1.0000x reference)
"""CropToBBox (crop_and_resize to 224x224 with bbox preprocessing) on 8 trn2 cores.

Strategy: data-parallel over N=64 images (8 per core). Bilinear crop+resize is
separable: out_c = Ry @ I_c @ Rx^T per channel, where Ry/Rx are [224, 512]
interpolation matrices with triangle (hat) weights relu(1 - |ys_i - h|),
masked to zero for out-of-range sample positions.

Device pipeline per image:
  stage 1: V^T_c[w, i] = sum_h I[h, w, c] * RyT[h, i]   (lhsT = image slice)
  stage 2: O_c[i, j]   = sum_w V^T_c[w, i] * RxT[w, j]
Both as exact float32 matmuls. Ry/Rx built
on-device by ScalarE from host-computed sample coordinates (2 ops per
128-chunk: u = Abs(ys - w), then Relu(-u + 1)).

Host computes ys/xs [224] per image in fp32 replicating the reference bbox
math bit-exactly; invalid (out-of-range) positions are set to -1e5 so all
triangle weights vanish (matches the reference's zero-fill masking).
"""

import numpy as np

N_FULL = 64
H = W = 512
C = 3
CH = CW = 224
NPAD = 224  # == CH/CW; fp32 matmuls have no min-N constraint
N_CORES = 8
PER_CORE = N_FULL // N_CORES
FACTOR = 1.2

_CACHE = {}


def _host_coords(threshold, bboxes):
    """Replicate process_bbox + crop_and_resize coordinate math in fp32."""
    f = np.float32
    th = np.asarray(threshold, f)
    bb = np.asarray(bboxes, f)
    default = np.array([0.0, 1.0, 0.0, 1.0], f)
    filt = np.where(th < f(0.5), default, bb).astype(f)
    x1, y1, x2, y2 = filt[:, 0], filt[:, 1], filt[:, 2], filt[:, 3]

    def resize_side(small, large):
        side = (large - small).astype(f)
        new_side = (side * f(FACTOR)).astype(f)
        center = ((small + large) / f(2)).astype(f)
        half = (new_side / f(2)).astype(f)
        new_min = np.clip((center - half).astype(f), f(0), f(1)).astype(f)
        new_max = np.clip((center + half).astype(f), f(0), f(1)).astype(f)
        return new_min, new_max

    nx1, nx2 = resize_side(x1, x2)
    ny1, ny2 = resize_side(y1, y2)
    # reference: boxes = stack([nx1, ny1, nx2, ny2]); crop uses [y1,x1,y2,x2]
    by1, bx1, by2, bx2 = nx1, ny1, nx2, ny2

    idx = np.arange(CH, dtype=f)
    ys = (by1[:, None] * f(H - 1)).astype(f) + (
        idx[None, :] * (((by2 - by1) * f(H - 1)).astype(f) / f(CH - 1)).astype(f)[:, None]
    ).astype(f)
    ys = ys.astype(f)
    xs = (bx1[:, None] * f(W - 1)).astype(f) + (
        idx[None, :] * (((bx2 - bx1) * f(W - 1)).astype(f) / f(CW - 1)).astype(f)[:, None]
    ).astype(f)
    xs = xs.astype(f)

    BAD = f(-1e5)
    ys = np.where((ys >= f(0)) & (ys <= f(H - 1)), ys, BAD).astype(f)
    xs = np.where((xs >= f(0)) & (xs <= f(W - 1)), xs, BAD).astype(f)

    ys_pad = np.full((N_FULL, NPAD), BAD, f)
    xs_pad = np.full((N_FULL, NPAD), BAD, f)
    ys_pad[:, :CH] = ys
    xs_pad[:, :CW] = xs
    return ys_pad, xs_pad


def _build_nc():
    from concourse import bacc, tile
    import concourse.mybir as mybir

    dt = mybir.dt
    F32 = dt.float32
    F32R = dt.float32r
    AF = mybir.ActivationFunctionType

    # Bacc (not raw Bass): its compile pipeline splits semaphore waits into
    # event-semaphore instructions, satisfying the TRN2 1-wait-per-instruction
    # ISA constraint that walrus codegen enforces.
    nc = bacc.Bacc(None, target_bir_lowering=False)
    images_d = nc.declare_dram_parameter("images", [PER_CORE, H, W, C], F32, isOutput=False)
    ys_d = nc.declare_dram_parameter("ys", [PER_CORE, NPAD], F32, isOutput=False)
    xs_d = nc.declare_dram_parameter("xs", [PER_CORE, NPAD], F32, isOutput=False)
    wneg_d = nc.declare_dram_parameter("wneg", [128, 4], F32, isOutput=False)
    out_d = nc.declare_dram_parameter("out", [PER_CORE, CH, CW, C], F32, isOutput=True)

    KH = H // 128  # 4 h-chunks
    KW = W // 128  # 4 w-chunks
    IC = 2         # i-chunks of 112
    ICH = CH // IC

    with tile.TileContext(nc) as tc:
        with (
            tc.tile_pool(name="const", bufs=1) as cpool,
            tc.tile_pool(name="img", bufs=3) as ipool,
            tc.tile_pool(name="wts", bufs=8) as wpool,
            tc.tile_pool(name="tmp", bufs=3) as tpool,
            tc.tile_pool(name="vt", bufs=24) as vpool,
            tc.tile_pool(name="outsb", bufs=3) as opool,
            tc.tile_pool(name="psv", bufs=4, space="PSUM") as psv_pool,
            tc.tile_pool(name="pso", bufs=3, space="PSUM") as pso_pool,
            tc.tile_pool(name="bc", bufs=2) as bcpool,
        ):
            # issue image 0's load first: it is the longest pole in the
            # pipeline fill; split it so the first h-chunks land early and
            # stage-1 matmuls can start while the rest streams in
            img0 = ipool.tile([128, KH, W, C], F32, name="img4_0", tag="img4")
            for lo, hi in ((0, 1), (1, 2), (2, 4)):
                nc.sync.dma_start(
                    out=img0[:, lo:hi],
                    in_=images_d[0, 128 * lo:128 * hi].rearrange(
                        "(kh p) w c -> p kh w c", p=128),
                )

            wneg = cpool.tile([128, 4], F32)
            nc.scalar.dma_start(out=wneg[:], in_=wneg_d[:])

            for n in range(PER_CORE):
                if n == 0:
                    img4 = img0
                else:
                    img4 = ipool.tile([128, KH, W, C], F32, name=f"img4_{n}", tag="img4")
                    nc.sync.dma_start(
                        out=img4[:],
                        in_=images_d[n].rearrange("(kh p) w c -> p kh w c", p=128),
                    )
                img = [img4[:, k] for k in range(KH)]

                # per-image coordinate rows (ScalarE HWDGE queue, off the
                # SP image-DMA path), then broadcast to 128 partitions on
                # GpSimd (otherwise idle)
                ysr = bcpool.tile([1, NPAD], F32, name=f"ysr_{n}", tag="ysr")
                nc.scalar.dma_start(out=ysr[:], in_=ys_d[n].unsqueeze(0))
                xsr = bcpool.tile([1, NPAD], F32, name=f"xsr_{n}", tag="xsr")
                nc.scalar.dma_start(out=xsr[:], in_=xs_d[n].unsqueeze(0))
                ysb = bcpool.tile([128, NPAD], F32, tag="ysb")
                nc.gpsimd.partition_broadcast(ysb[:], ysr[:])
                xsb = bcpool.tile([128, NPAD], F32, tag="xsb")
                nc.gpsimd.partition_broadcast(xsb[:], xsr[:])

                # interpolation weight chunks: [128, NPAD] per 128-row window
                ryt = []
                rxt = []
                for k in range(KH):
                    u = tpool.tile([128, NPAD], F32)
                    nc.scalar.activation(u[:], ysb[:], AF.Abs, bias=wneg[:, k:k + 1], scale=1.0)
                    r = wpool.tile([128, NPAD], F32, tag="ryt")
                    nc.scalar.activation(r[:], u[:], AF.Relu, bias=1.0, scale=-1.0)
                    ryt.append(r)
                for k in range(KW):
                    u2 = tpool.tile([128, NPAD], F32, tag="u")
                    nc.scalar.activation(u2[:], xsb[:], AF.Abs, bias=wneg[:, k:k + 1], scale=1.0)
                    r = wpool.tile([128, NPAD], F32, tag="rxt")
                    nc.scalar.activation(r[:], u2[:], AF.Relu, bias=1.0, scale=-1.0)
                    rxt.append(r)

                # stage 1: V^T_c[w_chunk][p=w, i] = sum_h I[h, w, c] RyT[h, i]
                vt = {}
                for ci in range(C):
                    for wk in range(KW):
                        pv = psv_pool.tile([128, NPAD], F32)
                        for kh in range(KH):
                            nc.tensor.matmul(
                                pv[:],
                                img[kh][:, wk * 128:(wk + 1) * 128, ci],
                                ryt[kh][:],
                                start=(kh == 0),
                                stop=(kh == KH - 1),
                            )
                        v = vpool.tile([128, CH], F32, tag="vt")
                        nc.vector.tensor_copy(v[:], pv[:, :CH])
                        vt[(ci, wk)] = v

                # stage 2 + channel interleave + store
                for ic in range(IC):
                    osb = opool.tile([ICH, CW, C], F32)
                    for ci in range(C):
                        po = pso_pool.tile([ICH, NPAD], F32)
                        for wk in range(KW):
                            nc.tensor.matmul(
                                po[:],
                                vt[(ci, wk)][:, ic * ICH:(ic + 1) * ICH],
                                rxt[wk][:],
                                start=(wk == 0),
                                stop=(wk == KW - 1),
                            )
                        nc.vector.tensor_copy(osb[:, :, ci], po[:, :CW])
                    nc.sync.dma_start(
                        out=out_d[n, ic * ICH:(ic + 1) * ICH], in_=osb[:]
                    )
    nc.finalize()
    return nc


def _get_nc():
    if "nc" not in _CACHE:
        _CACHE["nc"] = _build_nc()
    return _CACHE["nc"]


def _wneg_const():
    p = np.arange(128, dtype=np.float32)
    return np.stack([-(128.0 * k + p) for k in range(4)], axis=1).astype(np.float32)


def _ensure_device_platform():
    """If the process pinned jax to cpu (e.g. JAX_PLATFORMS=cpu), re-resolve
    backends so the 8 axon/neuron devices are visible for the PJRT run."""
    import jax
    try:
        if len([d for d in jax.devices() if d.platform != "cpu"]) >= N_CORES:
            return
    except Exception:
        pass
    import os
    os.environ.pop("JAX_PLATFORMS", None)
    try:
        jax.config.update("jax_platforms", None)
    except Exception:
        pass
    for clear in ("clear_backends",):
        try:
            getattr(jax, clear)()
            break
        except Exception:
            pass


def kernel(threshold, bboxes, images):
    from concourse.bass_utils import run_bass_kernel_spmd

    _ensure_device_platform()

    ys_pad, xs_pad = _host_coords(threshold, bboxes)
    images = np.ascontiguousarray(np.asarray(images, np.float32))
    wneg = _wneg_const()

    nc = _get_nc()
    in_maps = []
    for core in range(N_CORES):
        sl = slice(core * PER_CORE, (core + 1) * PER_CORE)
        in_maps.append({
            "images": images[sl],
            "ys": np.ascontiguousarray(ys_pad[sl]),
            "xs": np.ascontiguousarray(xs_pad[sl]),
            "wneg": wneg,
        })
    import os
    trace = bool(os.environ.get("CROP_TRACE"))
    if trace:
        try:
            import antenv.axon_hooks  # noqa: F401
        except ImportError:
            trace = False
    res = run_bass_kernel_spmd(nc, in_maps, list(range(N_CORES)), trace=trace)
    _CACHE["last_res"] = res
    out = np.concatenate([res.results[i]["out"] for i in range(N_CORES)], axis=0)
    return out.astype(np.float32)



# revision 8
# speedup vs baseline: 4.5469x; 4.5469x over previous
"""CropToBBox (crop_and_resize to 224x224 with bbox preprocessing) on 8 trn2 cores.

v2 strategy (vs full-image fp32 baseline):
  - Gather-DMA only the needed source window per image: rows [r0, r0+S) and a
    64px-aligned column window of K blocks, via SWDGE dma_gather with
    host-computed int16 indices (idx = 24*h + 3*cb0 in 256B units).
  - Host assigns images to 8 (core, slot) pairs so each slot has similar
    window sizes; per-slot (S, K) are compile-time (kernel is rebuilt if the
    size signature changes), indices/coords stay runtime inputs.
  - Separable bilinear resize as two matmul stages in float32r with 256-wide
    moving dims (full-rate on the PE vs 1/4 for plain fp32).
  - hat weights built on device from gpsimd-broadcast coords; negated-hat
    variant on DVE/gpsimd (sign cancels across the two stages), positive
    variant on ACT. Copies greedily load-balanced across DVE/ACT/gpsimd.
  - Output written planar [n, c, i, j]; host does the final NHWC transpose.
"""

import numpy as np

N_FULL = 64
H = W = 512
C = 3
CH = CW = 224
NPAD = 256           # padded free dim for fp32r full-rate matmuls
N_CORES = 8
PER_CORE = N_FULL // N_CORES
FACTOR = 1.2
BAD = np.float32(-1e5)

_CACHE = {}


def _host_coords(threshold, bboxes):
    """Replicate process_bbox + crop_and_resize coordinate math in fp32.

    Returns ys, xs [64, 224] with BAD at invalid (out-of-range) positions.
    """
    f = np.float32
    th = np.asarray(threshold, f)
    bb = np.asarray(bboxes, f)
    default = np.array([0.0, 1.0, 0.0, 1.0], f)
    filt = np.where(th < f(0.5), default, bb).astype(f)
    x1, y1, x2, y2 = filt[:, 0], filt[:, 1], filt[:, 2], filt[:, 3]

    def resize_side(small, large):
        side = (large - small).astype(f)
        new_side = (side * f(FACTOR)).astype(f)
        center = ((small + large) / f(2)).astype(f)
        half = (new_side / f(2)).astype(f)
        new_min = np.clip((center - half).astype(f), f(0), f(1)).astype(f)
        new_max = np.clip((center + half).astype(f), f(0), f(1)).astype(f)
        return new_min, new_max

    nx1, nx2 = resize_side(x1, x2)
    ny1, ny2 = resize_side(y1, y2)
    # reference: boxes = stack([nx1, ny1, nx2, ny2]); crop uses [y1,x1,y2,x2]
    by1, bx1, by2, bx2 = nx1, ny1, nx2, ny2

    idx = np.arange(CH, dtype=f)
    ys = (by1[:, None] * f(H - 1)).astype(f) + (
        idx[None, :] * (((by2 - by1) * f(H - 1)).astype(f) / f(CH - 1)).astype(f)[:, None]
    ).astype(f)
    ys = ys.astype(f)
    xs = (bx1[:, None] * f(W - 1)).astype(f) + (
        idx[None, :] * (((bx2 - bx1) * f(W - 1)).astype(f) / f(CW - 1)).astype(f)[:, None]
    ).astype(f)
    xs = xs.astype(f)

    ys = np.where((ys >= f(0)) & (ys <= f(H - 1)), ys, BAD).astype(f)
    xs = np.where((xs >= f(0)) & (xs <= f(W - 1)), xs, BAD).astype(f)
    return ys, xs


def _windows(ys, xs):
    """Per image: row window (r0, S) and 64px col-block window (cb0, K)."""
    out = []
    for n in range(N_FULL):
        yv = ys[n][ys[n] > -1e4]
        xv = xs[n][xs[n] > -1e4]
        if yv.size == 0 or xv.size == 0:
            out.append((0, 1, 0, 1))
            continue
        r0 = int(np.floor(yv.min())); r1 = int(np.ceil(yv.max()))
        r0 = max(0, min(r0, H - 1)); r1 = max(r0, min(r1, H - 1))
        c0 = int(np.floor(xv.min())); c1 = int(np.ceil(xv.max()))
        c0 = max(0, min(c0, W - 1)); c1 = max(c0, min(c1, W - 1))
        cb0 = c0 // 64
        K = c1 // 64 - cb0 + 1
        out.append((r0, r1 - r0 + 1, cb0, K))
    return out


def _slot_cost(S, K):
    """Rough per-slot ns cost: gather DMA + PE + vector-engine work."""
    Wpx = 64 * K
    Hc = -(-S // 128)
    Wc = -(-Wpx // 128)
    elem_b = K * 768
    per_desc = max(elem_b * (2.0 if elem_b < 512 else 1.0) / 22.5, 7.0)
    dma = S / 16.0 * per_desc
    pe = (3 * Wc * Hc + 6 * Wc) * 107.0
    vec = (Hc + Wc) * 2 * 340.0 + Wc * 950.0
    return dma + 0.65 * pe + 0.45 * vec


def _plan(wins):
    """Assign 64 images to 8 slots x 8 cores; returns perm and signature."""
    area = np.array([w[1] * w[3] for w in wins])
    order = np.argsort(-area, kind="stable")
    groups = [list(order[j * 8:(j + 1) * 8]) for j in range(8)]

    def gcost(g):
        S = max(wins[i][1] for i in g)
        K = max(wins[i][3] for i in g)
        return _slot_cost(S, K)

    for _ in range(6):
        improved = False
        for a in range(8):
            for b in range(a + 1, 8):
                base = gcost(groups[a]) + gcost(groups[b])
                best = None
                for ia in range(8):
                    for ib in range(8):
                        ga = groups[a][:]; gb = groups[b][:]
                        ga[ia], gb[ib] = gb[ib], ga[ia]
                        c = gcost(ga) + gcost(gb)
                        if c < base - 1e-9:
                            base = c; best = (ia, ib)
                if best is not None:
                    ia, ib = best
                    groups[a][ia], groups[b][ib] = groups[b][ib], groups[a][ia]
                    improved = True
        if not improved:
            break

    groups.sort(key=gcost)  # program order: small slots first
    sig = []
    perm = [[0] * 8 for _ in range(N_CORES)]
    for j, g in enumerate(groups):
        S = max(wins[i][1] for i in g)
        K = max(wins[i][3] for i in g)
        sig.append((S, K))
        for c, img in enumerate(g):
            perm[c][j] = img
    return perm, tuple(sig)


def _build_nc(sig):
    from concourse import bacc, tile
    from concourse import library_config
    import concourse.mybir as mybir
    import bass_rust

    dt = mybir.dt
    F32 = dt.float32
    F32R = dt.float32r
    I16 = dt.int16
    AF = mybir.ActivationFunctionType
    ALU = mybir.AluOpType

    slots = []
    idx_off = 0
    for S, K in sig:
        Hc = -(-S // 128)
        Wc = -(-(64 * K) // 128)
        cols = -(-S // 16)
        slots.append(dict(S=S, K=K, Hc=Hc, Wc=Wc, icols=cols, ioff=idx_off))
        idx_off += cols
    TOTC = idx_off

    nc = bacc.Bacc(None, target_bir_lowering=False)
    images_d = nc.declare_dram_parameter("images", [PER_CORE, H, W, C], F32, isOutput=False)
    meta_d = nc.declare_dram_parameter("meta", [1, 8 * 2 * NPAD], F32, isOutput=False)
    idxs_d = nc.declare_dram_parameter("idxs", [128, TOTC], I16, isOutput=False)
    wneg_d = nc.declare_dram_parameter("wneg", [128, 4], F32, isOutput=False)
    out_d = nc.declare_dram_parameter("out", [PER_CORE, C, CH, CW], F32, isOutput=True)

    # greedy engine load balance (ns estimates); gpsimd pre-charged with
    # gather desc-gen and the per-slot partition broadcasts
    load = {"vector": 0.0, "scalar": 0.0, "gpsimd": 0.0}
    load["gpsimd"] += sum(994 + 0.34 * s["S"] for s in slots) + 8 * 900.0

    def pick_copy(cost):
        cands = [("vector", cost), ("scalar", cost * 0.9 + 120.0),
                 ("gpsimd", cost * 1.9)]
        name, c = min(cands, key=lambda kv: load[kv[0]] + kv[1])
        load[name] += c
        return name

    with tile.TileContext(nc) as tc:
        with (
            tc.tile_pool(name="const", bufs=1) as cpool,
            tc.tile_pool(name="img", bufs=1) as ipool,
            tc.tile_pool(name="bc", bufs=3) as bcpool,
            tc.tile_pool(name="wts", bufs=1) as wpool,
            tc.tile_pool(name="tmp", bufs=3) as tpool,
            tc.tile_pool(name="vt", bufs=2) as vpool,
            tc.tile_pool(name="outsb", bufs=4) as opool,
            tc.tile_pool(name="ps1", bufs=3, space="PSUM") as ps1_pool,
            tc.tile_pool(name="ps1b", bufs=2, space="PSUM") as ps1b_pool,
            tc.tile_pool(name="ps2", bufs=3, space="PSUM") as ps2_pool,
        ):
            nc.gpsimd.load_library(library_config.mlp)

            meta_sb = cpool.tile([1, 8 * 2 * NPAD], F32)
            nc.scalar.dma_start(out=meta_sb[:], in_=meta_d[:])
            idx_sb = cpool.tile([128, TOTC], I16)
            nc.scalar.dma_start(out=idx_sb[:], in_=idxs_d[:])
            wneg = cpool.tile([128, 4], F32)
            nc.scalar.dma_start(out=wneg[:], in_=wneg_d[:])

            def copy_op(dst, src, cost):
                e = pick_copy(cost)
                if e == "scalar":
                    nc.scalar.activation(dst, src, AF.Copy, bias=0.0, scale=1.0)
                else:
                    getattr(nc, e).tensor_copy(dst, src)

            # issue all gathers up front
            ximg = []
            for j, s in enumerate(slots):
                S, K, Hc = s["S"], s["K"], s["Hc"]
                elem = K * 192
                xt = ipool.tile([128, Hc, elem], F32, name=f"X{j}", tag=f"X{j}")
                nrow = 12289 - 3 * K
                in_ap = bass_rust.AP(
                    tensor=images_d, offset=j * (H * W * C),
                    ap=[[64, nrow], [1, elem]],
                )
                nc.gpsimd.dma_gather(
                    xt[:], in_ap, idx_sb[:, s["ioff"]:s["ioff"] + s["icols"]],
                    S, S, elem, elem_step=64,
                )
                ximg.append(xt)

            for j, s in enumerate(slots):
                S, K, Hc, Wc = s["S"], s["K"], s["Hc"], s["Wc"]
                Wpx = 64 * K

                # broadcast ys''|xs'' [1, 512] -> [128, 512]
                bc = bcpool.tile([128, 2 * NPAD], F32, tag="bc")
                nc.gpsimd.partition_broadcast(
                    bc[:], meta_sb[0:1, j * 2 * NPAD:(j + 1) * 2 * NPAD])

                # weights: per-slot engine scheme (sign cancels across stages)
                wcost = (Hc + Wc) * 2 * 340.0
                cands = [("vector", wcost), ("scalar", wcost * 0.85 + 200.0),
                         ("gpsimd", wcost * 1.9)]
                weng, wc = min(cands, key=lambda kv: load[kv[0]] + kv[1])
                load[weng] += wc

                wtiles = []  # Hc ryt tiles then Wc rxt tiles
                for k in range(Hc + Wc):
                    if k < Hc:
                        src = bc[:, 0:NPAD]
                        kidx = k
                        rows = min(128, S - 128 * k)
                    else:
                        kidx = k - Hc
                        src = bc[:, NPAD:2 * NPAD]
                        rows = min(128, Wpx - 128 * kidx)
                    wt = wpool.tile([128, NPAD], F32, name=f"w{j}_{k}", tag=f"w{j}_{k}")
                    u = tpool.tile([128, NPAD], F32, tag="u")
                    if weng == "scalar":
                        nc.scalar.activation(
                            u[0:rows, :], src[0:rows, :], AF.Abs,
                            bias=wneg[0:rows, kidx:kidx + 1], scale=1.0,
                        )
                        nc.scalar.activation(
                            wt[0:rows, :], u[0:rows, :], AF.Relu,
                            bias=1.0, scale=-1.0,
                        )
                    else:
                        eng = getattr(nc, weng)
                        # u = |src - pos| ; wt = min(u - 1, 0) = -hat
                        eng.tensor_scalar(
                            out=u[0:rows, :], in0=src[0:rows, :],
                            scalar1=wneg[0:rows, kidx:kidx + 1], scalar2=0.0,
                            op0=ALU.add, op1=ALU.abs_max,
                        )
                        eng.tensor_scalar(
                            out=wt[0:rows, :], in0=u[0:rows, :],
                            scalar1=1.0, scalar2=0.0,
                            op0=ALU.subtract, op1=ALU.min,
                        )
                    wtiles.append(wt)

                xv = ximg[j][:].rearrange("p hc (w c) -> p hc w c", c=C)

                # stage 1: V[w, i] = sum_h img[h, w, c] * ryt[h, i]
                vts = []  # per wk: (v2 [128, 448] = ci0|ci1, v1 [128, 224] = ci2)
                for wk in range(Wc):
                    wseg = min(128, Wpx - 128 * wk)
                    pv2 = ps1_pool.tile([128, 2 * NPAD], F32, tag="pv2")
                    pv1 = ps1b_pool.tile([128, NPAD], F32, tag="pv1")
                    for ci in range(C):
                        dst = (pv2[0:wseg, ci * NPAD:(ci + 1) * NPAD] if ci < 2
                               else pv1[0:wseg, :])
                        for k in range(Hc):
                            rows = min(128, S - 128 * k)
                            nc.tensor.matmul(
                                dst,
                                xv[0:rows, k, 128 * wk:128 * wk + wseg, ci].bitcast(F32R),
                                wtiles[k][0:rows, :].bitcast(F32R),
                                start=(k == 0),
                                stop=(k == Hc - 1),
                            )
                    v2 = vpool.tile([128, 2 * CH], F32, tag=f"v2_{wk}")
                    v1 = vpool.tile([128, CH], F32, tag=f"v1_{wk}")
                    src2 = pv2[0:wseg, :].rearrange("p (c i) -> p c i", c=2)[:, :, 0:CH]
                    dst2 = v2[0:wseg, :].rearrange("p (c i) -> p c i", c=2)
                    copy_op(dst2, src2, 2 * CH * 1.04 + 130)
                    copy_op(v1[0:wseg, :], pv1[0:wseg, 0:CH], CH * 1.04 + 130)
                    vts.append((v2, v1))

                # stage 2: out[i, j'] = sum_w V[w, i] * rxt[w, j']
                for ci in range(C):
                    po = ps2_pool.tile([112, 2 * NPAD], F32, tag="po")
                    for ic in range(2):
                        dst = po[:, ic * NPAD:(ic + 1) * NPAD]
                        for wk in range(Wc):
                            wseg = min(128, Wpx - 128 * wk)
                            v2, v1 = vts[wk]
                            if ci < 2:
                                lhs = v2[0:wseg, ci * CH + ic * 112: ci * CH + ic * 112 + 112]
                            else:
                                lhs = v1[0:wseg, ic * 112:ic * 112 + 112]
                            nc.tensor.matmul(
                                dst,
                                lhs.bitcast(F32R),
                                wtiles[Hc + wk][0:wseg, :].bitcast(F32R),
                                start=(wk == 0),
                                stop=(wk == Wc - 1),
                            )
                    osb = opool.tile([112, 2 * CW], F32, tag="osb")
                    srco = po[:, :].rearrange("p (a jj) -> p a jj", a=2)[:, :, 0:CW]
                    dsto = osb[:, :].rearrange("p (a jj) -> p a jj", a=2)
                    copy_op(dsto, srco, 2 * CW * 1.04 + 130)
                    for ic in range(2):
                        nc.sync.dma_start(
                            out=out_d[j, ci, ic * 112:ic * 112 + 112, :],
                            in_=osb[:, ic * CW:(ic + 1) * CW],
                        )
    nc.finalize()
    nc._engine_load_estimate = dict(load)
    return nc


def _get_nc(sig):
    key = ("nc", sig)
    if key not in _CACHE:
        _CACHE[key] = _build_nc(sig)
    return _CACHE[key]


def _host_arrays(images, ys, xs, wins, perm, sig):
    """Build per-core input dicts for the signature."""
    f = np.float32
    slots = [(S, K, -(-S // 16)) for S, K in sig]
    TOTC = sum(c for _, _, c in slots)

    p = np.arange(128, dtype=f)
    wneg = np.stack([-(p + 128.0 * k) for k in range(4)], axis=1).astype(f)

    in_maps = []
    for core in range(N_CORES):
        imgs = np.empty((PER_CORE, H, W, C), f)
        meta = np.full((8, 2 * NPAD), BAD, f)  # reshaped to [1, 8*512] below
        idxs = np.zeros((16, TOTC), np.int16)
        off = 0
        for j, (S, K, cols) in enumerate(slots):
            n = perm[core][j]
            imgs[j] = images[n]
            r0, Sn, cb0, Kn = wins[n]
            cb0p = min(cb0, 8 - K)
            meta[j, 0:CH] = ys[n] - f(r0)
            meta[j, NPAD:NPAD + CW] = xs[n] - f(64 * cb0p)
            for t in range(S):
                h = min(r0 + t, H - 1)
                idxs[t % 16, off + t // 16] = 24 * h + 3 * cb0p
            off += cols
        in_maps.append({
            "images": imgs,
            "meta": meta.reshape(1, -1),
            "idxs": np.tile(idxs, (8, 1)),
            "wneg": wneg,
        })
    return in_maps


def _ensure_device_platform():
    import jax
    try:
        if len([d for d in jax.devices() if d.platform != "cpu"]) >= N_CORES:
            return
    except Exception:
        pass
    import os
    os.environ.pop("JAX_PLATFORMS", None)
    try:
        jax.config.update("jax_platforms", None)
    except Exception:
        pass
    for clear in ("clear_backends",):
        try:
            getattr(jax, clear)()
            break
        except Exception:
            pass


def prepare(threshold, bboxes, images):
    """Host-side planning shared by kernel() and the sim test."""
    ys, xs = _host_coords(threshold, bboxes)
    wins = _windows(ys, xs)
    perm, sig = _plan(wins)
    images = np.ascontiguousarray(np.asarray(images, np.float32))
    in_maps = _host_arrays(images, ys, xs, wins, perm, sig)
    return in_maps, perm, sig


def assemble(results, perm):
    """results[core]["out"] [8, 3, 224, 224] -> full [64, 224, 224, 3]."""
    full = np.empty((N_FULL, CH, CW, C), np.float32)
    for core in range(N_CORES):
        o = np.asarray(results[core]["out"])
        o = np.transpose(o, (0, 2, 3, 1))
        for j in range(8):
            full[perm[core][j]] = o[j]
    return full


def kernel(threshold, bboxes, images):
    from concourse.bass_utils import run_bass_kernel_spmd

    _ensure_device_platform()
    in_maps, perm, sig = prepare(threshold, bboxes, images)
    nc = _get_nc(sig)
    _CACHE["nc"] = nc

    import os
    trace = bool(os.environ.get("CROP_TRACE"))
    if trace:
        try:
            import antenv.axon_hooks  # noqa: F401
        except ImportError:
            trace = False
    res = run_bass_kernel_spmd(nc, in_maps, list(range(N_CORES)), trace=trace)
    _CACHE["last_res"] = res
    return assemble(res.results, perm).astype(np.float32)


# revision 13
# speedup vs baseline: 5.4020x; 1.1881x over previous
"""CropToBBox (crop_and_resize to 224x224 with bbox preprocessing) on 8 trn2 cores.

v2 strategy (vs full-image fp32 baseline):
  - Gather-DMA only the needed source window per image: rows [r0, r0+S) and a
    64px-aligned column window of K blocks, via SWDGE dma_gather with
    host-computed int16 indices (idx = 24*h + 3*cb0 in 256B units).
  - Host assigns images to 8 (core, slot) pairs so each slot has similar
    window sizes; per-slot (S, K) are compile-time (kernel is rebuilt if the
    size signature changes), indices/coords stay runtime inputs.
  - Separable bilinear resize as two matmul stages in float32r with 256-wide
    moving dims (full-rate on the PE vs 1/4 for plain fp32).
  - hat weights built on device from gpsimd-broadcast coords; negated-hat
    variant on DVE/gpsimd (sign cancels across the two stages), positive
    variant on ACT. Copies greedily load-balanced across DVE/ACT/gpsimd.
  - Output written planar [n, c, i, j]; host does the final NHWC transpose.
"""

import numpy as np

N_FULL = 64
H = W = 512
C = 3
CH = CW = 224
NPAD = 256           # padded free dim for fp32r full-rate matmuls
N_CORES = 8
PER_CORE = N_FULL // N_CORES
FACTOR = 1.2
BAD = np.float32(-1e5)

_CACHE = {}


def _host_coords(threshold, bboxes):
    """Replicate process_bbox + crop_and_resize coordinate math in fp32.

    Returns ys, xs [64, 224] with BAD at invalid (out-of-range) positions.
    """
    f = np.float32
    th = np.asarray(threshold, f)
    bb = np.asarray(bboxes, f)
    default = np.array([0.0, 1.0, 0.0, 1.0], f)
    filt = np.where(th < f(0.5), default, bb).astype(f)
    x1, y1, x2, y2 = filt[:, 0], filt[:, 1], filt[:, 2], filt[:, 3]

    def resize_side(small, large):
        side = (large - small).astype(f)
        new_side = (side * f(FACTOR)).astype(f)
        center = ((small + large) / f(2)).astype(f)
        half = (new_side / f(2)).astype(f)
        new_min = np.clip((center - half).astype(f), f(0), f(1)).astype(f)
        new_max = np.clip((center + half).astype(f), f(0), f(1)).astype(f)
        return new_min, new_max

    nx1, nx2 = resize_side(x1, x2)
    ny1, ny2 = resize_side(y1, y2)
    # reference: boxes = stack([nx1, ny1, nx2, ny2]); crop uses [y1,x1,y2,x2]
    by1, bx1, by2, bx2 = nx1, ny1, nx2, ny2

    idx = np.arange(CH, dtype=f)
    ys = (by1[:, None] * f(H - 1)).astype(f) + (
        idx[None, :] * (((by2 - by1) * f(H - 1)).astype(f) / f(CH - 1)).astype(f)[:, None]
    ).astype(f)
    ys = ys.astype(f)
    xs = (bx1[:, None] * f(W - 1)).astype(f) + (
        idx[None, :] * (((bx2 - bx1) * f(W - 1)).astype(f) / f(CW - 1)).astype(f)[:, None]
    ).astype(f)
    xs = xs.astype(f)

    ys = np.where((ys >= f(0)) & (ys <= f(H - 1)), ys, BAD).astype(f)
    xs = np.where((xs >= f(0)) & (xs <= f(W - 1)), xs, BAD).astype(f)
    return ys, xs


def _windows(ys, xs):
    """Per image: row window (r0, S) and 64px col-block window (cb0, K)."""
    out = []
    for n in range(N_FULL):
        yv = ys[n][ys[n] > -1e4]
        xv = xs[n][xs[n] > -1e4]
        if yv.size == 0 or xv.size == 0:
            out.append((0, 1, 0, 1))
            continue
        r0 = int(np.floor(yv.min())); r1 = int(np.ceil(yv.max()))
        r0 = max(0, min(r0, H - 1)); r1 = max(r0, min(r1, H - 1))
        c0 = int(np.floor(xv.min())); c1 = int(np.ceil(xv.max()))
        c0 = max(0, min(c0, W - 1)); c1 = max(c0, min(c1, W - 1))
        cb0 = c0 // 64
        K = c1 // 64 - cb0 + 1
        out.append((r0, r1 - r0 + 1, cb0, K))
    return out


def _slot_cost(S, K):
    """Rough per-slot ns cost: gather DMA + PE + vector-engine work."""
    Wpx = 64 * K
    Hc = -(-S // 128)
    Wc = -(-Wpx // 128)
    elem_b = K * 768
    per_desc = max(elem_b * (2.0 if elem_b < 512 else 1.0) / 22.5, 7.0)
    dma = S / 16.0 * per_desc
    pe = (3 * Wc * Hc + 6 * Wc) * 107.0
    vec = (Hc + Wc) * 2 * 340.0 + Wc * 950.0
    return dma + 0.65 * pe + 0.45 * vec


def _plan(wins):
    """Assign 64 images to 8 slots x 8 cores; returns perm and signature."""
    area = np.array([w[1] * w[3] for w in wins])
    order = np.argsort(-area, kind="stable")
    groups = [list(order[j * 8:(j + 1) * 8]) for j in range(8)]

    def gcost(g):
        S = max(wins[i][1] for i in g)
        K = max(wins[i][3] for i in g)
        return _slot_cost(S, K)

    for _ in range(6):
        improved = False
        for a in range(8):
            for b in range(a + 1, 8):
                base = gcost(groups[a]) + gcost(groups[b])
                best = None
                for ia in range(8):
                    for ib in range(8):
                        ga = groups[a][:]; gb = groups[b][:]
                        ga[ia], gb[ib] = gb[ib], ga[ia]
                        c = gcost(ga) + gcost(gb)
                        if c < base - 1e-9:
                            base = c; best = (ia, ib)
                if best is not None:
                    ia, ib = best
                    groups[a][ia], groups[b][ib] = groups[b][ib], groups[a][ia]
                    improved = True
        if not improved:
            break

    groups.sort(key=gcost)  # program order: small slots first
    sig = []
    perm = [[0] * 8 for _ in range(N_CORES)]
    for j, g in enumerate(groups):
        S = max(wins[i][1] for i in g)
        K = max(wins[i][3] for i in g)
        sig.append((S, K))
        for c, img in enumerate(g):
            perm[c][j] = img
    return perm, tuple(sig)


def _build_nc(sig):
    from concourse import bacc, tile
    from concourse import library_config
    import concourse.mybir as mybir
    import bass_rust

    dt = mybir.dt
    F32 = dt.float32
    F32R = dt.float32r
    I16 = dt.int16
    AF = mybir.ActivationFunctionType
    ALU = mybir.AluOpType

    slots = []
    idx_off = 0
    for S, K in sig:
        Hc = -(-S // 128)
        Wc = -(-(64 * K) // 128)
        cols = -(-S // 16)
        slots.append(dict(S=S, K=K, Hc=Hc, Wc=Wc, icols=cols, ioff=idx_off))
        idx_off += cols
    TOTC = idx_off

    nc = bacc.Bacc(None, target_bir_lowering=False)
    images_d = nc.declare_dram_parameter("images", [PER_CORE, H, W, C], F32, isOutput=False)
    meta_d = nc.declare_dram_parameter("meta", [1, 8 * 2 * NPAD], F32, isOutput=False)
    idxs_d = nc.declare_dram_parameter("idxs", [128, TOTC], I16, isOutput=False)
    wneg_d = nc.declare_dram_parameter("wneg", [128, 4], F32, isOutput=False)
    out_d = nc.declare_dram_parameter("out", [PER_CORE, C, CH, CW], F32, isOutput=True)

    # greedy engine load balance (ns estimates); gpsimd pre-charged with
    # gather desc-gen and the per-slot partition broadcasts
    load = {"vector": 0.0, "scalar": 0.0, "gpsimd": 0.0}
    load["gpsimd"] += sum(994 + 0.34 * s["S"] for s in slots) + 8 * 450.0

    def op_cost(eng, free):
        if eng == "vector":
            return free * 1.04 + 160.0
        if eng == "scalar":
            return free * 0.833 + 370.0
        return free * 1.39 + 130.0  # gpsimd

    def pick_copy(free):
        name = min(load, key=lambda e: load[e] + op_cost(e, free))
        load[name] += op_cost(name, free)
        return name

    with tile.TileContext(nc) as tc:
        with (
            tc.tile_pool(name="const", bufs=1) as cpool,
            tc.tile_pool(name="img", bufs=1) as ipool,
            tc.tile_pool(name="bc", bufs=3) as bcpool,
            tc.tile_pool(name="wts", bufs=1) as wpool,
            tc.tile_pool(name="tmp", bufs=3) as tpool,
            tc.tile_pool(name="vt", bufs=2) as vpool,
            tc.tile_pool(name="outsb", bufs=4) as opool,
            tc.tile_pool(name="ps1", bufs=3, space="PSUM") as ps1_pool,
            tc.tile_pool(name="ps1b", bufs=2, space="PSUM") as ps1b_pool,
            tc.tile_pool(name="ps2", bufs=3, space="PSUM") as ps2_pool,
        ):
            nc.gpsimd.load_library(library_config.mlp)

            meta_sb = cpool.tile([1, 8 * 2 * NPAD], F32)
            nc.sync.dma_start(out=meta_sb[:], in_=meta_d[:])
            idx_sb = cpool.tile([128, TOTC], I16)
            nc.sync.dma_start(out=idx_sb[:], in_=idxs_d[:])
            wneg = cpool.tile([128, 4], F32)
            nc.sync.dma_start(out=wneg[:], in_=wneg_d[:])

            def copy_op(dst, src, free):
                e = pick_copy(free)
                if e == "scalar":
                    nc.scalar.activation(dst, src, AF.Copy, bias=0.0, scale=1.0)
                else:
                    getattr(nc, e).tensor_copy(dst, src)

            # issue all gathers up front
            ximg = []
            for j, s in enumerate(slots):
                S, K, Hc = s["S"], s["K"], s["Hc"]
                elem = K * 192
                xt = ipool.tile([128, Hc, elem], F32, name=f"X{j}", tag=f"X{j}")
                nrow = 12289 - 3 * K
                in_ap = bass_rust.AP(
                    tensor=images_d, offset=j * (H * W * C),
                    ap=[[64, nrow], [1, elem]],
                )
                nc.gpsimd.dma_gather(
                    xt[:], in_ap, idx_sb[:, s["ioff"]:s["ioff"] + s["icols"]],
                    S, S, elem, elem_step=64,
                )
                ximg.append(xt)

            for j, s in enumerate(slots):
                S, K, Hc, Wc = s["S"], s["K"], s["Hc"], s["Wc"]
                Wpx = 64 * K

                # broadcast ys''|xs'' [1, 512] -> [128, 512]
                bc = bcpool.tile([128, 2 * NPAD], F32, tag="bc")
                nc.gpsimd.partition_broadcast(
                    bc[:], meta_sb[0:1, j * 2 * NPAD:(j + 1) * 2 * NPAD])

                # weights: per-slot engine scheme (sign cancels across stages)
                nops = (Hc + Wc) * 2
                weng = min(load, key=lambda e: load[e] + nops * op_cost(e, NPAD))
                load[weng] += nops * op_cost(weng, NPAD)

                wtiles = []  # Hc ryt tiles then Wc rxt tiles
                for k in range(Hc + Wc):
                    if k < Hc:
                        src = bc[:, 0:NPAD]
                        kidx = k
                        rows = min(128, S - 128 * k)
                    else:
                        kidx = k - Hc
                        src = bc[:, NPAD:2 * NPAD]
                        rows = min(128, Wpx - 128 * kidx)
                    wt = wpool.tile([128, NPAD], F32, name=f"w{j}_{k}", tag=f"w{j}_{k}")
                    u = tpool.tile([128, NPAD], F32, tag="u")
                    if weng == "scalar":
                        nc.scalar.activation(
                            u[0:rows, :], src[0:rows, :], AF.Abs,
                            bias=wneg[0:rows, kidx:kidx + 1], scale=1.0,
                        )
                        nc.scalar.activation(
                            wt[0:rows, :], u[0:rows, :], AF.Relu,
                            bias=1.0, scale=-1.0,
                        )
                    else:
                        eng = getattr(nc, weng)
                        # u = |src - pos| ; wt = min(u - 1, 0) = -hat
                        eng.tensor_scalar(
                            out=u[0:rows, :], in0=src[0:rows, :],
                            scalar1=wneg[0:rows, kidx:kidx + 1], scalar2=0.0,
                            op0=ALU.add, op1=ALU.abs_max,
                        )
                        eng.tensor_scalar(
                            out=wt[0:rows, :], in0=u[0:rows, :],
                            scalar1=1.0, scalar2=0.0,
                            op0=ALU.subtract, op1=ALU.min,
                        )
                    wtiles.append(wt)

                xv = ximg[j][:].rearrange("p hc (w c) -> p hc w c", c=C)

                # stage 1: V[w, i] = sum_h img[h, w, c] * ryt[h, i]
                vts = []  # per wk: (v2 [128, 448] = ci0|ci1, v1 [128, 224] = ci2)
                for wk in range(Wc):
                    wseg = min(128, Wpx - 128 * wk)
                    pv2 = ps1_pool.tile([128, 2 * NPAD], F32, tag="pv2")
                    pv1 = ps1b_pool.tile([128, NPAD], F32, tag="pv1")
                    for ci in range(C):
                        dst = (pv2[0:wseg, ci * NPAD:(ci + 1) * NPAD] if ci < 2
                               else pv1[0:wseg, :])
                        for k in range(Hc):
                            rows = min(128, S - 128 * k)
                            nc.tensor.matmul(
                                dst,
                                xv[0:rows, k, 128 * wk:128 * wk + wseg, ci].bitcast(F32R),
                                wtiles[k][0:rows, :].bitcast(F32R),
                                start=(k == 0),
                                stop=(k == Hc - 1),
                            )
                    v2 = vpool.tile([128, 2 * CH], F32, tag=f"v2_{wk}")
                    v1 = vpool.tile([128, CH], F32, tag=f"v1_{wk}")
                    src2 = pv2[0:wseg, :].rearrange("p (c i) -> p c i", c=2)[:, :, 0:CH]
                    dst2 = v2[0:wseg, :].rearrange("p (c i) -> p c i", c=2)
                    copy_op(dst2, src2, 2 * CH)
                    copy_op(v1[0:wseg, :], pv1[0:wseg, 0:CH], CH)
                    vts.append((v2, v1))

                # stage 2: out[i, j'] = sum_w V[w, i] * rxt[w, j']
                osb = opool.tile([112, C * 2 * CW], F32, tag="osb")
                for ci in range(C):
                    po = ps2_pool.tile([112, 2 * NPAD], F32, tag="po")
                    for ic in range(2):
                        dst = po[:, ic * NPAD:(ic + 1) * NPAD]
                        for wk in range(Wc):
                            wseg = min(128, Wpx - 128 * wk)
                            v2, v1 = vts[wk]
                            if ci < 2:
                                lhs = v2[0:wseg, ci * CH + ic * 112: ci * CH + ic * 112 + 112]
                            else:
                                lhs = v1[0:wseg, ic * 112:ic * 112 + 112]
                            nc.tensor.matmul(
                                dst,
                                lhs.bitcast(F32R),
                                wtiles[Hc + wk][0:wseg, :].bitcast(F32R),
                                start=(wk == 0),
                                stop=(wk == Wc - 1),
                            )
                    srco = po[:, :].rearrange("p (a jj) -> p a jj", a=2)[:, :, 0:CW]
                    dsto = osb[:, ci * 2 * CW:(ci + 1) * 2 * CW].rearrange(
                        "p (a jj) -> p a jj", a=2)
                    copy_op(dsto, srco, 2 * CW)
                # one DMA per slot: [p, (ci, ic, jj)] -> planar [ci, ic*112+p, jj]
                nc.sync.dma_start(
                    out=out_d[j].rearrange("c (ic p) jj -> p c ic jj", ic=2),
                    in_=osb[:].rearrange("p (c ic jj) -> p c ic jj", c=C, ic=2),
                )
    nc.finalize()
    nc._engine_load_estimate = dict(load)
    return nc


def _get_nc(sig):
    key = ("nc", sig)
    if key not in _CACHE:
        _CACHE[key] = _build_nc(sig)
    return _CACHE[key]


def _host_arrays(images, ys, xs, wins, perm, sig):
    """Build per-core input dicts for the signature."""
    f = np.float32
    slots = [(S, K, -(-S // 16)) for S, K in sig]
    TOTC = sum(c for _, _, c in slots)

    p = np.arange(128, dtype=f)
    wneg = np.stack([-(p + 128.0 * k) for k in range(4)], axis=1).astype(f)

    in_maps = []
    for core in range(N_CORES):
        imgs = np.empty((PER_CORE, H, W, C), f)
        meta = np.full((8, 2 * NPAD), BAD, f)  # reshaped to [1, 8*512] below
        idxs = np.zeros((16, TOTC), np.int16)
        off = 0
        for j, (S, K, cols) in enumerate(slots):
            n = perm[core][j]
            imgs[j] = images[n]
            r0, Sn, cb0, Kn = wins[n]
            cb0p = min(cb0, 8 - K)
            meta[j, 0:CH] = ys[n] - f(r0)
            meta[j, NPAD:NPAD + CW] = xs[n] - f(64 * cb0p)
            for t in range(S):
                h = min(r0 + t, H - 1)
                idxs[t % 16, off + t // 16] = 24 * h + 3 * cb0p
            off += cols
        in_maps.append({
            "images": imgs,
            "meta": meta.reshape(1, -1),
            "idxs": np.tile(idxs, (8, 1)),
            "wneg": wneg,
        })
    return in_maps


def _ensure_device_platform():
    import jax
    try:
        if len([d for d in jax.devices() if d.platform != "cpu"]) >= N_CORES:
            return
    except Exception:
        pass
    import os
    os.environ.pop("JAX_PLATFORMS", None)
    try:
        jax.config.update("jax_platforms", None)
    except Exception:
        pass
    for clear in ("clear_backends",):
        try:
            getattr(jax, clear)()
            break
        except Exception:
            pass


def prepare(threshold, bboxes, images):
    """Host-side planning shared by kernel() and the sim test."""
    ys, xs = _host_coords(threshold, bboxes)
    wins = _windows(ys, xs)
    perm, sig = _plan(wins)
    images = np.ascontiguousarray(np.asarray(images, np.float32))
    in_maps = _host_arrays(images, ys, xs, wins, perm, sig)
    return in_maps, perm, sig


def assemble(results, perm):
    """results[core]["out"] [8, 3, 224, 224] -> full [64, 224, 224, 3]."""
    full = np.empty((N_FULL, CH, CW, C), np.float32)
    for core in range(N_CORES):
        o = np.asarray(results[core]["out"])
        o = np.transpose(o, (0, 2, 3, 1))
        for j in range(8):
            full[perm[core][j]] = o[j]
    return full


def kernel(threshold, bboxes, images):
    from concourse.bass_utils import run_bass_kernel_spmd

    _ensure_device_platform()
    in_maps, perm, sig = prepare(threshold, bboxes, images)
    nc = _get_nc(sig)
    _CACHE["nc"] = nc

    import os
    trace = bool(os.environ.get("CROP_TRACE"))
    if trace:
        try:
            import antenv.axon_hooks  # noqa: F401
        except ImportError:
            trace = False
    res = run_bass_kernel_spmd(nc, in_maps, list(range(N_CORES)), trace=trace)
    _CACHE["last_res"] = res
    return assemble(res.results, perm).astype(np.float32)


# revision 20
# speedup vs baseline: 5.7049x; 1.0561x over previous
"""CropToBBox (crop_and_resize to 224x224 with bbox preprocessing) on 8 trn2 cores.

v2 strategy (vs full-image fp32 baseline):
  - Gather-DMA only the needed source window per image: rows [r0, r0+S) and a
    64px-aligned column window of K blocks, via SWDGE dma_gather with
    host-computed int16 indices (idx = 24*h + 3*cb0 in 256B units).
  - Host assigns images to 8 (core, slot) pairs so each slot has similar
    window sizes; per-slot (S, K) are compile-time (kernel is rebuilt if the
    size signature changes), indices/coords stay runtime inputs.
  - Separable bilinear resize as two matmul stages in float32r with 256-wide
    moving dims (full-rate on the PE vs 1/4 for plain fp32).
  - hat weights built on device from gpsimd-broadcast coords; negated-hat
    variant on DVE/gpsimd (sign cancels across the two stages), positive
    variant on ACT. Copies greedily load-balanced across DVE/ACT/gpsimd.
  - Output written planar [n, c, i, j]; host does the final NHWC transpose.
"""

import numpy as np

N_FULL = 64
H = W = 512
C = 3
CH = CW = 224
NPAD = 256           # padded free dim for fp32r full-rate matmuls
N_CORES = 8
PER_CORE = N_FULL // N_CORES
FACTOR = 1.2
BAD = np.float32(-1e5)

_CACHE = {}


def _host_coords(threshold, bboxes):
    """Replicate process_bbox + crop_and_resize coordinate math in fp32.

    Returns ys, xs [64, 224] with BAD at invalid (out-of-range) positions.
    """
    f = np.float32
    th = np.asarray(threshold, f)
    bb = np.asarray(bboxes, f)
    default = np.array([0.0, 1.0, 0.0, 1.0], f)
    filt = np.where(th < f(0.5), default, bb).astype(f)
    x1, y1, x2, y2 = filt[:, 0], filt[:, 1], filt[:, 2], filt[:, 3]

    def resize_side(small, large):
        side = (large - small).astype(f)
        new_side = (side * f(FACTOR)).astype(f)
        center = ((small + large) / f(2)).astype(f)
        half = (new_side / f(2)).astype(f)
        new_min = np.clip((center - half).astype(f), f(0), f(1)).astype(f)
        new_max = np.clip((center + half).astype(f), f(0), f(1)).astype(f)
        return new_min, new_max

    nx1, nx2 = resize_side(x1, x2)
    ny1, ny2 = resize_side(y1, y2)
    # reference: boxes = stack([nx1, ny1, nx2, ny2]); crop uses [y1,x1,y2,x2]
    by1, bx1, by2, bx2 = nx1, ny1, nx2, ny2

    idx = np.arange(CH, dtype=f)
    ys = (by1[:, None] * f(H - 1)).astype(f) + (
        idx[None, :] * (((by2 - by1) * f(H - 1)).astype(f) / f(CH - 1)).astype(f)[:, None]
    ).astype(f)
    ys = ys.astype(f)
    xs = (bx1[:, None] * f(W - 1)).astype(f) + (
        idx[None, :] * (((bx2 - bx1) * f(W - 1)).astype(f) / f(CW - 1)).astype(f)[:, None]
    ).astype(f)
    xs = xs.astype(f)

    ys = np.where((ys >= f(0)) & (ys <= f(H - 1)), ys, BAD).astype(f)
    xs = np.where((xs >= f(0)) & (xs <= f(W - 1)), xs, BAD).astype(f)
    return ys, xs


def _windows(ys, xs):
    """Per image: row window (r0, S) and 64px col-block window (cb0, K)."""
    out = []
    for n in range(N_FULL):
        yv = ys[n][ys[n] > -1e4]
        xv = xs[n][xs[n] > -1e4]
        if yv.size == 0 or xv.size == 0:
            out.append((0, 1, 0, 1))
            continue
        r0 = int(np.floor(yv.min())); r1 = int(np.ceil(yv.max()))
        r0 = max(0, min(r0, H - 1)); r1 = max(r0, min(r1, H - 1))
        c0 = int(np.floor(xv.min())); c1 = int(np.ceil(xv.max()))
        c0 = max(0, min(c0, W - 1)); c1 = max(c0, min(c1, W - 1))
        cb0 = c0 // 64
        K = c1 // 64 - cb0 + 1
        out.append((r0, r1 - r0 + 1, cb0, K))
    return out


def _slot_cost(S, K):
    """Rough per-slot ns cost: gather DMA + PE + vector-engine work."""
    Wpx = 64 * K
    Hc = -(-S // 128)
    Wc = -(-Wpx // 128)
    elem_b = K * 768
    per_desc = max(elem_b * (2.0 if elem_b < 512 else 1.0) / 22.5, 7.0)
    dma = S / 16.0 * per_desc
    pe = (3 * Wc * Hc + 6 * Wc) * 107.0
    vec = (Hc + Wc) * 2 * 340.0 + Wc * 950.0
    return dma + 0.65 * pe + 0.45 * vec


def _plan(wins):
    """Assign 64 images to 8 slots x 8 cores; returns perm and signature."""
    area = np.array([w[1] * w[3] for w in wins])
    order = np.argsort(-area, kind="stable")
    groups = [list(order[j * 8:(j + 1) * 8]) for j in range(8)]

    def gcost(g):
        S = max(wins[i][1] for i in g)
        K = max(wins[i][3] for i in g)
        return _slot_cost(S, K)

    for _ in range(6):
        improved = False
        for a in range(8):
            for b in range(a + 1, 8):
                base = gcost(groups[a]) + gcost(groups[b])
                best = None
                for ia in range(8):
                    for ib in range(8):
                        ga = groups[a][:]; gb = groups[b][:]
                        ga[ia], gb[ib] = gb[ib], ga[ia]
                        c = gcost(ga) + gcost(gb)
                        if c < base - 1e-9:
                            base = c; best = (ia, ib)
                if best is not None:
                    ia, ib = best
                    groups[a][ia], groups[b][ib] = groups[b][ib], groups[a][ia]
                    improved = True
        if not improved:
            break

    groups.sort(key=gcost)  # program order: small slots first
    sig = []
    perm = [[0] * 8 for _ in range(N_CORES)]
    for j, g in enumerate(groups):
        S = max(wins[i][1] for i in g)
        K = max(wins[i][3] for i in g)
        sig.append((S, K))
        for c, img in enumerate(g):
            perm[c][j] = img
    return perm, tuple(sig)


def _build_nc(sig):
    from concourse import bacc, tile
    from concourse import library_config
    import concourse.mybir as mybir
    import bass_rust

    dt = mybir.dt
    F32 = dt.float32
    F32R = dt.float32r
    I16 = dt.int16
    AF = mybir.ActivationFunctionType
    ALU = mybir.AluOpType

    slots = []
    idx_off = 0
    for S, K in sig:
        Hc = -(-S // 128)
        Wc = -(-(64 * K) // 128)
        cols = -(-S // 16)
        slots.append(dict(S=S, K=K, Hc=Hc, Wc=Wc, icols=cols, ioff=idx_off))
        idx_off += cols
    TOTC = idx_off

    nc = bacc.Bacc(None, target_bir_lowering=False)
    images_d = nc.declare_dram_parameter("images", [PER_CORE, H, W, C], F32, isOutput=False)
    meta_d = nc.declare_dram_parameter("meta", [8, 2 * NPAD], F32, isOutput=False)
    idxs_d = nc.declare_dram_parameter("idxs", [128, TOTC], I16, isOutput=False)
    wneg_d = nc.declare_dram_parameter("wneg", [128, 4], F32, isOutput=False)
    out_d = nc.declare_dram_parameter("out", [PER_CORE, C, CH, CW], F32, isOutput=True)

    # greedy engine load balance (ns estimates); gpsimd pre-charged with
    # gather desc-gen; sync participates only in DMA issue
    load = {"vector": 0.0, "scalar": 0.0, "gpsimd": 0.0, "sync": 0.0}
    load["gpsimd"] += sum(994 + 0.34 * s["S"] for s in slots)

    def op_cost(eng, free):
        if eng == "vector":
            return free * 1.04 + 160.0
        if eng == "scalar":
            return free * 0.833 + 370.0
        return free * 1.39 + 130.0  # gpsimd

    def pick_copy(free):
        cands = ["vector", "scalar", "gpsimd"]
        name = min(cands, key=lambda e: load[e] + op_cost(e, free))
        load[name] += op_cost(name, free)
        return name

    def pick_dma(bytes_per_partition):
        # HWDGE dma cost: ~0.385 ns per byte of partition line on the engine
        cost = bytes_per_partition * 0.385 + 150.0
        cands = ["sync", "scalar"]
        name = min(cands, key=lambda e: load[e] + cost)
        load[name] += cost
        return name

    with tile.TileContext(nc) as tc:
        with (
            tc.tile_pool(name="const", bufs=1) as cpool,
            tc.tile_pool(name="img", bufs=1) as ipool,
            tc.tile_pool(name="bc", bufs=3) as bcpool,
            tc.tile_pool(name="wts", bufs=1) as wpool,
            tc.tile_pool(name="tmp", bufs=3) as tpool,
            tc.tile_pool(name="vt", bufs=2) as vpool,
            tc.tile_pool(name="outsb", bufs=4) as opool,
            tc.tile_pool(name="ps1", bufs=3, space="PSUM") as ps1_pool,
            tc.tile_pool(name="ps1b", bufs=2, space="PSUM") as ps1b_pool,
            tc.tile_pool(name="ps2", bufs=3, space="PSUM") as ps2_pool,
        ):
            nc.gpsimd.load_library(library_config.mlp)

            idx_sb = cpool.tile([128, TOTC], I16)
            nc.sync.dma_start(out=idx_sb[:], in_=idxs_d[:])
            wneg = cpool.tile([128, 4], F32)
            nc.sync.dma_start(out=wneg[:], in_=wneg_d[:])

            def copy_op(dst, src, free):
                e = pick_copy(free)
                if e == "scalar":
                    nc.scalar.activation(dst, src, AF.Copy, bias=0.0, scale=1.0)
                else:
                    getattr(nc, e).tensor_copy(dst, src)

            # issue all gathers up front
            ximg = []
            for j, s in enumerate(slots):
                S, K, Hc = s["S"], s["K"], s["Hc"]
                elem = K * 192
                xt = ipool.tile([128, Hc, elem], F32, name=f"X{j}", tag=f"X{j}")
                nrow = 12289 - 3 * K
                in_ap = bass_rust.AP(
                    tensor=images_d, offset=j * (H * W * C),
                    ap=[[64, nrow], [1, elem]],
                )
                nc.gpsimd.dma_gather(
                    xt[:], in_ap, idx_sb[:, s["ioff"]:s["ioff"] + s["icols"]],
                    S, S, elem, elem_step=64,
                )
                ximg.append(xt)

            for j, s in enumerate(slots):
                S, K, Hc, Wc = s["S"], s["K"], s["Hc"], s["Wc"]
                Wpx = 64 * K

                # broadcast ys''|xs'' to [128, 512] via stride-0 dram DMA
                bc = bcpool.tile([128, 2 * NPAD], F32, tag="bc")
                e = pick_dma(2 * NPAD * 4)
                getattr(nc, e).dma_start(
                    out=bc[:],
                    in_=meta_d[j].unsqueeze(0).broadcast_to([128, 2 * NPAD]),
                )

                # weights: per-slot engine scheme (sign cancels across stages)
                nops = (Hc + Wc) * 2
                weng = min(load, key=lambda e: load[e] + nops * op_cost(e, NPAD))
                load[weng] += nops * op_cost(weng, NPAD)

                wtiles = []  # Hc ryt tiles then Wc rxt tiles
                for k in range(Hc + Wc):
                    if k < Hc:
                        src = bc[:, 0:NPAD]
                        kidx = k
                        rows = min(128, S - 128 * k)
                    else:
                        kidx = k - Hc
                        src = bc[:, NPAD:2 * NPAD]
                        rows = min(128, Wpx - 128 * kidx)
                    wt = wpool.tile([128, NPAD], F32, name=f"w{j}_{k}", tag=f"w{j}_{k}")
                    u = tpool.tile([128, NPAD], F32, tag="u")
                    if weng == "scalar":
                        nc.scalar.activation(
                            u[0:rows, :], src[0:rows, :], AF.Abs,
                            bias=wneg[0:rows, kidx:kidx + 1], scale=1.0,
                        )
                        nc.scalar.activation(
                            wt[0:rows, :], u[0:rows, :], AF.Relu,
                            bias=1.0, scale=-1.0,
                        )
                    else:
                        eng = getattr(nc, weng)
                        # u = |src - pos| ; wt = min(u - 1, 0) = -hat
                        eng.tensor_scalar(
                            out=u[0:rows, :], in0=src[0:rows, :],
                            scalar1=wneg[0:rows, kidx:kidx + 1], scalar2=0.0,
                            op0=ALU.add, op1=ALU.abs_max,
                        )
                        eng.tensor_scalar(
                            out=wt[0:rows, :], in0=u[0:rows, :],
                            scalar1=1.0, scalar2=0.0,
                            op0=ALU.subtract, op1=ALU.min,
                        )
                    wtiles.append(wt)

                xv = ximg[j][:].rearrange("p hc (w c) -> p hc w c", c=C)

                # stage 1: V[w, i] = sum_h img[h, w, c] * ryt[h, i]
                vts = []  # per wk: (v2 [128, 448] = ci0|ci1, v1 [128, 224] = ci2)
                for wk in range(Wc):
                    wseg = min(128, Wpx - 128 * wk)
                    pv2 = ps1_pool.tile([128, 2 * NPAD], F32, tag="pv2")
                    pv1 = ps1b_pool.tile([128, NPAD], F32, tag="pv1")
                    for ci in range(C):
                        dst = (pv2[0:wseg, ci * NPAD:(ci + 1) * NPAD] if ci < 2
                               else pv1[0:wseg, :])
                        for k in range(Hc):
                            rows = min(128, S - 128 * k)
                            nc.tensor.matmul(
                                dst,
                                xv[0:rows, k, 128 * wk:128 * wk + wseg, ci].bitcast(F32R),
                                wtiles[k][0:rows, :].bitcast(F32R),
                                start=(k == 0),
                                stop=(k == Hc - 1),
                            )
                    v2 = vpool.tile([128, 2 * CH], F32, tag=f"v2_{wk}")
                    v1 = vpool.tile([128, CH], F32, tag=f"v1_{wk}")
                    src2 = pv2[0:wseg, :].rearrange("p (c i) -> p c i", c=2)[:, :, 0:CH]
                    dst2 = v2[0:wseg, :].rearrange("p (c i) -> p c i", c=2)
                    copy_op(dst2, src2, 2 * CH)
                    copy_op(v1[0:wseg, :], pv1[0:wseg, 0:CH], CH)
                    vts.append((v2, v1))

                # stage 2: out[i, j'] = sum_w V[w, i] * rxt[w, j']
                osb = opool.tile([112, C * 2 * CW], F32, tag="osb")
                for ci in range(C):
                    po = ps2_pool.tile([112, 2 * NPAD], F32, tag="po")
                    for ic in range(2):
                        dst = po[:, ic * NPAD:(ic + 1) * NPAD]
                        for wk in range(Wc):
                            wseg = min(128, Wpx - 128 * wk)
                            v2, v1 = vts[wk]
                            if ci < 2:
                                lhs = v2[0:wseg, ci * CH + ic * 112: ci * CH + ic * 112 + 112]
                            else:
                                lhs = v1[0:wseg, ic * 112:ic * 112 + 112]
                            nc.tensor.matmul(
                                dst,
                                lhs.bitcast(F32R),
                                wtiles[Hc + wk][0:wseg, :].bitcast(F32R),
                                start=(wk == 0),
                                stop=(wk == Wc - 1),
                            )
                    srco = po[:, :].rearrange("p (a jj) -> p a jj", a=2)[:, :, 0:CW]
                    dsto = osb[:, ci * 2 * CW:(ci + 1) * 2 * CW].rearrange(
                        "p (a jj) -> p a jj", a=2)
                    copy_op(dsto, srco, 2 * CW)
                # per-channel output DMAs, spread across HWDGE engines:
                # [p, (ic, jj)] -> planar [ci, ic*112+p, jj]
                for ci in range(C):
                    e = pick_dma(2 * CW * 4)
                    getattr(nc, e).dma_start(
                        out=out_d[j, ci].rearrange("(ic p) jj -> p ic jj", ic=2),
                        in_=osb[:, ci * 2 * CW:(ci + 1) * 2 * CW].rearrange(
                            "p (ic jj) -> p ic jj", ic=2),
                    )
    nc.finalize()
    nc._engine_load_estimate = dict(load)
    return nc


def _get_nc(sig):
    key = ("nc", sig)
    if key not in _CACHE:
        _CACHE[key] = _build_nc(sig)
    return _CACHE[key]


def _host_arrays(images, ys, xs, wins, perm, sig):
    """Build per-core input dicts for the signature."""
    f = np.float32
    slots = [(S, K, -(-S // 16)) for S, K in sig]
    TOTC = sum(c for _, _, c in slots)

    p = np.arange(128, dtype=f)
    wneg = np.stack([-(p + 128.0 * k) for k in range(4)], axis=1).astype(f)

    in_maps = []
    for core in range(N_CORES):
        imgs = np.empty((PER_CORE, H, W, C), f)
        meta = np.full((8, 2 * NPAD), BAD, f)  # reshaped to [1, 8*512] below
        idxs = np.zeros((16, TOTC), np.int16)
        off = 0
        for j, (S, K, cols) in enumerate(slots):
            n = perm[core][j]
            imgs[j] = images[n]
            r0, Sn, cb0, Kn = wins[n]
            cb0p = min(cb0, 8 - K)
            meta[j, 0:CH] = ys[n] - f(r0)
            meta[j, NPAD:NPAD + CW] = xs[n] - f(64 * cb0p)
            for t in range(S):
                h = min(r0 + t, H - 1)
                idxs[t % 16, off + t // 16] = 24 * h + 3 * cb0p
            off += cols
        in_maps.append({
            "images": imgs,
            "meta": meta,
            "idxs": np.tile(idxs, (8, 1)),
            "wneg": wneg,
        })
    return in_maps


def _ensure_device_platform():
    import jax
    try:
        if len([d for d in jax.devices() if d.platform != "cpu"]) >= N_CORES:
            return
    except Exception:
        pass
    import os
    os.environ.pop("JAX_PLATFORMS", None)
    try:
        jax.config.update("jax_platforms", None)
    except Exception:
        pass
    for clear in ("clear_backends",):
        try:
            getattr(jax, clear)()
            break
        except Exception:
            pass


def prepare(threshold, bboxes, images):
    """Host-side planning shared by kernel() and the sim test."""
    ys, xs = _host_coords(threshold, bboxes)
    wins = _windows(ys, xs)
    perm, sig = _plan(wins)
    images = np.ascontiguousarray(np.asarray(images, np.float32))
    in_maps = _host_arrays(images, ys, xs, wins, perm, sig)
    return in_maps, perm, sig


def assemble(results, perm):
    """results[core]["out"] [8, 3, 224, 224] -> full [64, 224, 224, 3]."""
    full = np.empty((N_FULL, CH, CW, C), np.float32)
    for core in range(N_CORES):
        o = np.asarray(results[core]["out"])
        o = np.transpose(o, (0, 2, 3, 1))
        for j in range(8):
            full[perm[core][j]] = o[j]
    return full


def kernel(threshold, bboxes, images):
    from concourse.bass_utils import run_bass_kernel_spmd

    _ensure_device_platform()
    in_maps, perm, sig = prepare(threshold, bboxes, images)
    nc = _get_nc(sig)
    _CACHE["nc"] = nc

    import os
    trace = bool(os.environ.get("CROP_TRACE"))
    if trace:
        try:
            import antenv.axon_hooks  # noqa: F401
        except ImportError:
            trace = False
    res = run_bass_kernel_spmd(nc, in_maps, list(range(N_CORES)), trace=trace)
    _CACHE["last_res"] = res
    return assemble(res.results, perm).astype(np.float32)
